# revision 1
# baseline (speedup 1.0000x reference)
"""CenterLoss forward on 8 Trainium2 NeuronCores.

Reference semantics:
    distmat[b, c] = ||x_b||^2 + ||center_c||^2 - 2 <x_b, center_c>
    loss = sum(clip(distmat * onehot(labels), 1e-12, 1e12)) / B

The masked matrix is zero everywhere except (b, labels[b]), and clip() lifts
each of the B*(C-1) zeros to exactly 1e-12.  So:

    loss = ( sum_b clip(||x_b - centers[labels[b]]||^2, 1e-12, 1e12)
             + B*(C-1)*1e-12 ) / B

which needs only a row gather + per-row squared distance, not the full
(B, C) distance matrix (42 GFLOP -> ~4 MFLOP).

Device kernel (raw Bass, single basic block, SPMD data-parallel over batch):
  - centers are baked into the NEFF as a Const tensor (they are module
    *state* in the reference nn.Module); the runtime DMAs them to HBM at
    model-load time, so per-execution I/O is just the x shard + labels.
  - per core: 512 rows = 4 chunks of 128 partitions
      gpsimd:  label load, then 4 indirect-DMA row gathers
               centers[labels] -> SBUF (alternating two SWDGE queues),
               plus a tiny trailing dummy DMA that flushes the last
               gather's completion receipt through the lane promptly
      sync (HWDGE): the 4 x-chunk loads, one sem per DMA
      vector (DVE): subtract, fused square+row-reduce
               (scalar_tensor_tensor accum_out), clip
  - sync rules learned the hard way (sim race detector + hardware):
      * SWDGE/HWDGE descriptors complete out of order across rings; a
        semaphore value only proves HOW MANY of its increments landed,
        so every DMA whose completion matters gets its own semaphore
        (or a dedicated per-chunk one).
      * SWDGE sems may not be shared with HWDGE DMAs (must start at 0).
      * same-engine RAW on DVE needs an explicit sem edge.
  - per-core output: [128, 4] clipped per-row distances; host sums in
    f64, adds the analytic clip floor B*(C-1)*1e-12, divides by B.
"""

import hashlib
from contextlib import ExitStack

import numpy as np

import concourse.bass as bass
from concourse import mybir
from concourse.bass_utils import run_bass_kernel_spmd

B = 4096
D = 512
C = 10000
NCORES = 8
BL = B // NCORES          # 512 rows per core
P = 128                   # partitions
NT = BL // P              # 4 chunks per core

F32 = mybir.dt.float32
I32 = mybir.dt.int32

_CACHE = {}


def legalize_waits(nc, max_waits=1):
    """The walrus build in this container accepts at most one embedded
    sem-wait per TPB instruction ("Too many sync wait commands" otherwise).
    Split any excess into standalone single-wait InstEventSemaphore no-ops
    immediately before the instruction on the same engine — engine program
    order then enforces the identical synchronization."""
    n_split = 0
    for f in nc.m.functions:
        for b in f.blocks:
            insts = list(b.instructions)
            out = []
            for inst in insts:
                si = inst.sync_info
                waits = list(si.on_wait) if (si is not None and si.on_wait) else []
                if len(waits) > max_waits:
                    keep = waits[-max_waits:]
                    spill = waits[:-max_waits]
                    for k, w in enumerate(spill):
                        out.append(
                            mybir.InstEventSemaphore(
                                name=f"{inst.name}-lw{k}",
                                engine=inst.engine,
                                sync_info=mybir.SyncInfo(on_wait=[w], on_update=[]),
                            )
                        )
                        n_split += 1
                    inst.sync_info = mybir.SyncInfo(
                        on_wait=keep, on_update=list(si.on_update or [])
                    )
                out.append(inst)
            b.instructions = out
    return n_split


def build_nc(centers_np):
    nc = bass.Bass(num_swdge_queues=2)

    x = nc.dram_tensor("x", [BL, D], F32, kind="ExternalInput")
    # labels pre-arranged on host: [p, t] = original label[t*128 + p]
    labels = nc.dram_tensor("labels", [P, NT], I32, kind="ExternalInput")
    out = nc.dram_tensor("out", [P, NT], F32, kind="ExternalOutput")
    centers = nc.inline_tensor(
        np.ascontiguousarray(centers_np, dtype=np.float32), name="centers"
    )

    es = ExitStack()
    idx_sb = es.enter_context(nc.sbuf_tensor("idx_sb", [P, NT], I32))
    x_sb = es.enter_context(nc.sbuf_tensor("x_sb", [P, NT * D], F32))
    c_sb = es.enter_context(nc.sbuf_tensor("c_sb", [P, NT * D], F32))
    df_sb = es.enter_context(nc.sbuf_tensor("df_sb", [P, NT * D], F32))
    sq_sb = es.enter_context(nc.sbuf_tensor("sq_sb", [P, NT * D], F32))
    dcols = es.enter_context(nc.sbuf_tensor("dcols", [P, NT], F32))
    dclip = es.enter_context(nc.sbuf_tensor("dclip", [P, NT], F32))
    scr_sb = es.enter_context(nc.sbuf_tensor("scr_sb", [P, NT], I32))
    scr2_sb = es.enter_context(nc.sbuf_tensor("scr2_sb", [P, NT], I32))
    idx_sem = es.enter_context(nc.semaphore("idx_sem"))
    c_sems = [es.enter_context(nc.semaphore(f"c_sem{t}")) for t in range(NT)]
    xc_sems = [es.enter_context(nc.semaphore(f"xc_sem{t}")) for t in range(NT)]
    v_sem = es.enter_context(nc.semaphore("v_sem"))
    o_sem = es.enter_context(nc.semaphore("o_sem"))
    dve_sem = es.enter_context(nc.semaphore("dve_sem"))
    f_sem = es.enter_context(nc.semaphore("f_sem"))

    # ---- gpsimd: labels, then the gathers ----
    nc.gpsimd.dma_start(out=idx_sb[:, :], in_=labels[:, :]).then_inc(idx_sem, 16)
    # dummy DMA right behind the label load: the lane processes it next,
    # which pushes the label DMA's completion receipt through promptly
    # (otherwise idx_sem fires ~2 us late while the lane idles)
    nc.gpsimd.dma_start(out=scr2_sb[:, :], in_=labels[:, :]).then_inc(f_sem, 16)
    # ---- sync/HWDGE: x chunks in parallel with the above ----
    for t in range(NT):
        nc.sync.dma_start(
            out=x_sb[:, t * D:(t + 1) * D], in_=x[t * P:(t + 1) * P, :]
        ).then_inc(xc_sems[t], 16)
    nc.gpsimd.wait_ge(idx_sem, 16)  # indices resident before gathers
    gather_insts = []
    for t in range(NT):
        gi = nc.gpsimd.indirect_dma_start(
            out=c_sb[:, t * D:(t + 1) * D],
            out_offset=None,
            in_=centers[:],
            in_offset=bass.IndirectOffsetOnAxis(ap=idx_sb[:, t:t + 1], axis=0),
        ).then_inc(c_sems[t], 16)
        gather_insts.append(gi)
    # trailing dummy SWDGE DMA: flushes the last gather's completion receipt
    nc.gpsimd.dma_start(out=scr_sb[:, :], in_=labels[:, :]).then_inc(f_sem, 16)

    # ---- vector: per-chunk subtract + fused square/row-reduce ----
    n_dve = 0
    for t in range(NT):
        cs = slice(t * D, (t + 1) * D)
        nc.vector.wait_ge(xc_sems[t], 16)
        nc.vector.wait_ge(c_sems[t], 16)
        nc.vector.tensor_tensor(
            out=df_sb[:, cs],
            in0=x_sb[:, cs],
            in1=c_sb[:, cs],
            op=mybir.AluOpType.subtract,
        ).then_inc(dve_sem, 1)
        n_dve += 1
        nc.vector.wait_ge(dve_sem, n_dve)
        nc.vector.scalar_tensor_tensor(
            out=sq_sb[:, cs],
            in0=df_sb[:, cs],
            scalar=1.0,
            in1=df_sb[:, cs],
            op0=mybir.AluOpType.mult,
            op1=mybir.AluOpType.mult,
            accum_out=dcols[:, t:t + 1],
        ).then_inc(dve_sem, 1)
        n_dve += 1
    nc.vector.wait_ge(dve_sem, n_dve)
    nc.vector.tensor_scalar(
        out=dclip[:, :],
        in0=dcols[:, :],
        scalar1=1e-12,
        scalar2=1e12,
        op0=mybir.AluOpType.max,
        op1=mybir.AluOpType.min,
    ).then_inc(v_sem, 1)

    # ---- result out; runtime drains rings before reading outputs ----
    nc.gpsimd.wait_ge(v_sem, 1)
    nc.gpsimd.dma_start(out=out[:, :], in_=dclip[:, :]).then_inc(o_sem, 16)

    # alternate gathers across the two SWDGE queues
    for t, gi in enumerate(gather_insts):
        if t % 2 == 1:
            gi.ins.queue = "qPoolDynamic1"

    # NOTE: the ExitStack is intentionally NOT closed — closing would free
    # the semaphores and emit an expensive end-of-program drain + barrier;
    # Bass already clears the whole sem range in its preamble, so repeated
    # executions stay safe without it.
    legalize_waits(nc)
    return nc


def _get_nc(centers_np):
    arr = np.ascontiguousarray(centers_np, np.float32)
    key = hashlib.md5(arr.tobytes()).hexdigest()
    if _CACHE.get("key") != key:
        _CACHE["nc"] = build_nc(arr)
        _CACHE["key"] = key
    return _CACHE["nc"]


def make_in_maps(x, labels, centers=None):
    x = np.ascontiguousarray(np.asarray(x, dtype=np.float32))
    # [p, t] = label[t*128 + p] within each core's 512-row shard
    labels_i32 = np.ascontiguousarray(
        np.asarray(labels).astype(np.int32).reshape(NCORES, NT, P).transpose(0, 2, 1)
    )
    xs = x.reshape(NCORES, BL, D)
    return [{"x": xs[i], "labels": labels_i32[i]} for i in range(NCORES)]


def finalize(results):
    total = 0.0
    for r in results:
        total += float(np.asarray(r["out"], dtype=np.float64).sum())
    loss = (total + B * (C - 1) * 1e-12) / B
    return np.array(loss, dtype=np.float32)


def kernel(x, labels, centers):
    nc = _get_nc(centers)
    in_maps = make_in_maps(x, labels)
    res = run_bass_kernel_spmd(nc, in_maps, core_ids=list(range(NCORES)))
    return finalize(res.results)



# revision 13
# speedup vs baseline: 1.0215x; 1.0215x over previous
"""CenterLoss forward on 8 Trainium2 NeuronCores.

Reference semantics:
    distmat[b, c] = ||x_b||^2 + ||center_c||^2 - 2 <x_b, center_c>
    loss = sum(clip(distmat * onehot(labels), 1e-12, 1e12)) / B

The masked matrix is zero everywhere except (b, labels[b]), and clip() lifts
each of the B*(C-1) zeros to exactly 1e-12.  So:

    loss = ( sum_b clip(||x_b - centers[labels[b]]||^2, 1e-12, 1e12)
             + B*(C-1)*1e-12 ) / B

which needs only a row gather + per-row squared distance, not the full
(B, C) distance matrix (42 GFLOP -> ~4 MFLOP).

Device kernel (raw Bass, single basic block, SPMD data-parallel over batch),
v2 — restructured from the first working version using the trace + the
SWDGE cost model (994 ns fixed + 0.34 ns/descriptor per DMA instruction):

  - centers are baked into the NEFF as a Const tensor in bf16, augmented
    with an extra column holding -||c||^2/2 (rows padded to 520 cols for
    16B-aligned gather descriptors).  Gathering 513 cols of a row delivers
    both the center AND its squared norm in one descriptor; bf16 halves the
    gather's HBM traffic (532 KB/core vs 1 MB).
  - label load goes out FIRST on the sync/HWDGE queue (HWDGE setup ~0.6 us
    vs SWDGE ~1 us, and it no longer queues behind gpsimd preamble), then
    the 4 x-chunk loads on the same queue.
  - per core, 512 rows = 4 chunks of 128 partitions = 4 indirect-DMA
    gathers alternating across the two SWDGE queues.  (HW-verified dead
    ends: indirect_dma_start silently consumes only ONE index per
    partition, so a [128, 2] offset AP gathers half the rows; dma_gather
    would do all 512 in one instruction but needs a Q7 library reload
    that this walrus build cannot compile.)  A trailing dummy SWDGE DMA
    per queue flushes the gathers' completion receipts promptly.
  - DVE does ONE fused 513-col pass per chunk (scalar_tensor_tensor;
    tensor_tensor_reduce would fold the ||x||^2 init in too, but this
    walrus build rejects that opcode with "ISA wrong length"):
        out  = (c_aug * -2) * x_ext,   acc = sum(out)
    where x_ext carries a 1.0 in col 512 so the product term contributes
    (-2)*(-||c||^2/2)*1.0 = +||c||^2, making acc = ||c||^2 - 2<x,c>.
    ||x||^2 per chunk comes from a square-accumulate issued while the
    gathers are still in flight (DVE is otherwise idle there), and one
    final [128, 4] tensor_tensor add finishes the distances.
  - result [128, 4] goes back via the idle sync/HWDGE queue; the clip and
    the analytic floor B*(C-1)*1e-12 are applied host-side along with the
    final sum (host already owned the cross-core reduction).
  - sync rules kept from v1 (sim race detector + hardware):
      * every DMA whose completion matters gets its own semaphore;
      * SWDGE sems are never shared with HWDGE DMAs;
      * same-engine RAW on DVE gets an explicit sem edge (dve_sem chain).
"""

import hashlib
from contextlib import ExitStack

import ml_dtypes
import numpy as np

import concourse.bass as bass
from concourse import mybir
from concourse.bass_utils import run_bass_kernel_spmd

B = 4096
D = 512
C = 10000
NCORES = 8
BL = B // NCORES          # 512 rows per core
P = 128                   # partitions
NT = BL // P              # 4 chunks per core

CW = 520                  # bf16 cols per baked centers row (16B-aligned stride)
GW = D + 1                # gathered cols per row: center + (-|c|^2/2)
XW = D + 4                # f32 cols per x_ext chunk (col D = 1.0, 16B stride)

F32 = mybir.dt.float32
BF16 = mybir.dt.bfloat16
I32 = mybir.dt.int32

_CACHE = {}


def legalize_waits(nc, max_waits=1):
    """The walrus build in this container accepts at most one embedded
    sem-wait per TPB instruction ("Too many sync wait commands" otherwise).
    Split any excess into standalone single-wait InstEventSemaphore no-ops
    immediately before the instruction on the same engine — engine program
    order then enforces the identical synchronization."""
    n_split = 0
    for f in nc.m.functions:
        for b in f.blocks:
            insts = list(b.instructions)
            out = []
            for inst in insts:
                si = inst.sync_info
                waits = list(si.on_wait) if (si is not None and si.on_wait) else []
                if len(waits) > max_waits:
                    keep = waits[-max_waits:]
                    spill = waits[:-max_waits]
                    for k, w in enumerate(spill):
                        out.append(
                            mybir.InstEventSemaphore(
                                name=f"{inst.name}-lw{k}",
                                engine=inst.engine,
                                sync_info=mybir.SyncInfo(on_wait=[w], on_update=[]),
                            )
                        )
                        n_split += 1
                    inst.sync_info = mybir.SyncInfo(
                        on_wait=keep, on_update=list(si.on_update or [])
                    )
                out.append(inst)
            b.instructions = out
    return n_split


def make_caug(centers_np):
    """bf16 [C, CW]: cols 0..D-1 = centers, col D = -||c||^2/2, rest 0."""
    c64 = np.asarray(centers_np, dtype=np.float64)
    csq = (c64 * c64).sum(axis=1)
    caug = np.zeros((C, CW), dtype=np.float32)
    caug[:, :D] = centers_np
    caug[:, D] = (-0.5 * csq).astype(np.float32)
    return np.ascontiguousarray(caug.astype(ml_dtypes.bfloat16))


def build_nc(centers_np):
    nc = bass.Bass(num_swdge_queues=2)

    x = nc.dram_tensor("x", [BL, D], F32, kind="ExternalInput")
    # labels pre-arranged on host: [p, t] = original label[t*128 + p]
    labels = nc.dram_tensor("labels", [P, NT], I32, kind="ExternalInput")
    out = nc.dram_tensor("out", [P, NT], F32, kind="ExternalOutput")
    caug = nc.inline_tensor(make_caug(centers_np), name="caug")

    es = ExitStack()
    idx_sb = es.enter_context(nc.sbuf_tensor("idx_sb", [P, NT], I32))
    x_ext = es.enter_context(nc.sbuf_tensor("x_ext", [P, NT * XW], F32))
    c_sb = es.enter_context(nc.sbuf_tensor("c_sb", [P, NT * GW], BF16))
    sq_sb = es.enter_context(nc.sbuf_tensor("sq_sb", [P, NT * D], F32))
    prod_sb = es.enter_context(nc.sbuf_tensor("prod_sb", [P, NT * GW], F32))
    xsq_sb = es.enter_context(nc.sbuf_tensor("xsq_sb", [P, NT], F32))
    acc_sb = es.enter_context(nc.sbuf_tensor("acc_sb", [P, NT], F32))
    dist_sb = es.enter_context(nc.sbuf_tensor("dist_sb", [P, NT], F32))
    scr_sb = es.enter_context(nc.sbuf_tensor("scr_sb", [P, NT], I32))
    scr2_sb = es.enter_context(nc.sbuf_tensor("scr2_sb", [P, NT], I32))
    idx_sem = es.enter_context(nc.semaphore("idx_sem"))
    xc_sems = [es.enter_context(nc.semaphore(f"xc_sem{t}")) for t in range(NT)]
    c_sems = [es.enter_context(nc.semaphore(f"c_sem{t}")) for t in range(NT)]
    v_sem = es.enter_context(nc.semaphore("v_sem"))
    o_sem = es.enter_context(nc.semaphore("o_sem"))
    dve_sem = es.enter_context(nc.semaphore("dve_sem"))
    f_sem = es.enter_context(nc.semaphore("f_sem"))

    # ---- sync/HWDGE: labels first (they gate the gathers), then x chunks ----
    nc.sync.dma_start(out=idx_sb[:, :], in_=labels[:, :]).then_inc(idx_sem, 16)
    for t in range(NT):
        nc.sync.dma_start(
            out=x_ext[:, t * XW:t * XW + D], in_=x[t * P:(t + 1) * P, :]
        ).then_inc(xc_sems[t], 16)

    # ---- gpsimd: four 128-row gathers alternating across SWDGE queues ----
    nc.gpsimd.wait_ge(idx_sem, 16)
    gather_insts = []
    for t in range(NT):
        gi = nc.gpsimd.indirect_dma_start(
            out=c_sb[:, t * GW:(t + 1) * GW],
            out_offset=None,
            in_=caug[:],
            in_offset=bass.IndirectOffsetOnAxis(ap=idx_sb[:, t:t + 1], axis=0),
        ).then_inc(c_sems[t], 16)
        if t % 2 == 1:
            gi.ins.queue = "qPoolDynamic1"
        gather_insts.append(gi)
    # trailing dummy SWDGE DMAs: flush each queue's gather completion receipt
    nc.gpsimd.dma_start(out=scr_sb[:, :], in_=labels[:, :]).then_inc(f_sem, 16)
    d1 = nc.gpsimd.dma_start(out=scr2_sb[:, :], in_=labels[:, :]).then_inc(f_sem, 16)
    d1.ins.queue = "qPoolDynamic1"

    # ---- vector (DVE) ----
    n_dve = 0
    # x_ext's 1.0 column (disjoint from the DMA'd cols, no ordering needed)
    for t in range(NT):
        nc.vector.memset(x_ext[:, t * XW + D:t * XW + D + 1], 1.0).then_inc(dve_sem, 1)
        n_dve += 1
    # ||x||^2 per chunk while the gathers are still in flight
    for t in range(NT):
        xc = x_ext[:, t * XW:t * XW + D]
        nc.vector.wait_ge(xc_sems[t], 16)
        nc.vector.scalar_tensor_tensor(
            out=sq_sb[:, t * D:(t + 1) * D],
            in0=xc,
            scalar=1.0,
            in1=xc,
            op0=mybir.AluOpType.mult,
            op1=mybir.AluOpType.mult,
            accum_out=xsq_sb[:, t:t + 1],
        ).then_inc(dve_sem, 1)
        n_dve += 1
    # fused per-chunk partial distance: acc = sum(-2 * c_aug * x_ext)
    #                                       = ||c||^2 - 2<x, c>
    n_pre = n_dve  # memsets + squares done (RAW edges for x_ext col D)
    for t in range(NT):
        nc.vector.wait_ge(c_sems[t], 16)
        nc.vector.wait_ge(dve_sem, n_pre)
        nc.vector.scalar_tensor_tensor(
            out=prod_sb[:, t * GW:(t + 1) * GW],
            in0=c_sb[:, t * GW:(t + 1) * GW],
            scalar=-2.0,
            in1=x_ext[:, t * XW:t * XW + GW],
            op0=mybir.AluOpType.mult,
            op1=mybir.AluOpType.mult,
            accum_out=acc_sb[:, t:t + 1],
        ).then_inc(dve_sem, 1)
        n_dve += 1
    # dist = acc + ||x||^2
    nc.vector.wait_ge(dve_sem, n_dve)
    nc.vector.tensor_tensor(
        out=dist_sb[:, :],
        in0=acc_sb[:, :],
        in1=xsq_sb[:, :],
        op=mybir.AluOpType.add,
    ).then_inc(v_sem, 1)

    # ---- result out via the idle sync/HWDGE queue ----
    nc.sync.wait_ge(v_sem, 1)
    nc.sync.dma_start(out=out[:, :], in_=dist_sb[:, :]).then_inc(o_sem, 16)

    # NOTE: the ExitStack is intentionally NOT closed — closing would free
    # the semaphores and emit an expensive end-of-program drain + barrier;
    # the NEFF-level postamble already clears the kernel sem range, so
    # repeated executions stay safe without it.
    legalize_waits(nc)
    return nc


def _get_nc(centers_np):
    arr = np.ascontiguousarray(centers_np, np.float32)
    key = hashlib.md5(arr.tobytes()).hexdigest()
    if _CACHE.get("key") != key:
        _CACHE["nc"] = build_nc(arr)
        _CACHE["key"] = key
    return _CACHE["nc"]


def make_in_maps(x, labels, centers=None):
    x = np.ascontiguousarray(np.asarray(x, dtype=np.float32))
    # [p, t] = label[t*128 + p] within each core's 512-row shard
    labels_i32 = np.ascontiguousarray(
        np.asarray(labels).astype(np.int32).reshape(NCORES, NT, P).transpose(0, 2, 1)
    )
    xs = x.reshape(NCORES, BL, D)
    return [{"x": xs[i], "labels": labels_i32[i]} for i in range(NCORES)]


def finalize(results):
    total = 0.0
    for r in results:
        d = np.asarray(r["out"], dtype=np.float64)
        total += float(np.clip(d, 1e-12, 1e12).sum())
    loss = (total + B * (C - 1) * 1e-12) / B
    return np.array(loss, dtype=np.float32)


def kernel(x, labels, centers):
    nc = _get_nc(centers)
    in_maps = make_in_maps(x, labels)
    res = run_bass_kernel_spmd(nc, in_maps, core_ids=list(range(NCORES)))
    return finalize(res.results)


# revision 15
# speedup vs baseline: 1.0880x; 1.0651x over previous
"""CenterLoss forward on 8 Trainium2 NeuronCores.

Reference semantics:
    distmat[b, c] = ||x_b||^2 + ||center_c||^2 - 2 <x_b, center_c>
    loss = sum(clip(distmat * onehot(labels), 1e-12, 1e12)) / B

The masked matrix is zero everywhere except (b, labels[b]), and clip() lifts
each of the B*(C-1) zeros to exactly 1e-12.  So:

    loss = ( sum_b clip(||x_b - centers[labels[b]]||^2, 1e-12, 1e12)
             + B*(C-1)*1e-12 ) / B

which needs only a row gather + per-row squared distance, not the full
(B, C) distance matrix (42 GFLOP -> ~4 MFLOP).

Device kernel (raw Bass, single basic block, SPMD data-parallel over batch),
v2 — restructured from the first working version using the trace + the
SWDGE cost model (994 ns fixed + 0.34 ns/descriptor per DMA instruction):

  - centers are baked into the NEFF as a Const tensor in bf16, augmented
    with an extra column holding -||c||^2/2 (rows padded to 520 cols for
    16B-aligned gather descriptors).  Gathering 513 cols of a row delivers
    both the center AND its squared norm in one descriptor; bf16 halves the
    gather's HBM traffic (532 KB/core vs 1 MB).
  - label load goes out FIRST on the sync/HWDGE queue (HWDGE setup ~0.6 us
    vs SWDGE ~1 us, and it no longer queues behind gpsimd preamble), then
    the 4 x-chunk loads on the same queue.
  - per core, 512 rows = 4 chunks of 128 partitions = 4 indirect-DMA
    gathers alternating across the two SWDGE queues.  (HW-verified dead
    ends: indirect_dma_start silently consumes only ONE index per
    partition, so a [128, 2] offset AP gathers half the rows; dma_gather
    would do all 512 in one instruction but needs a Q7 library reload
    that this walrus build cannot compile.)  A trailing dummy SWDGE DMA
    per queue flushes the gathers' completion receipts promptly.
  - DVE does ONE fused 513-col pass per chunk (scalar_tensor_tensor;
    tensor_tensor_reduce would fold the ||x||^2 init in too, but this
    walrus build rejects that opcode with "ISA wrong length"):
        out  = (c_aug * -2) * x_ext,   acc = sum(out)
    where x_ext carries a 1.0 in col 512 so the product term contributes
    (-2)*(-||c||^2/2)*1.0 = +||c||^2, making acc = ||c||^2 - 2<x,c>.
    ||x||^2 per chunk comes from a square-accumulate issued while the
    gathers are still in flight (DVE is otherwise idle there), and one
    final [128, 4] tensor_tensor add finishes the distances.
  - result [128, 4] goes back via the idle sync/HWDGE queue; the clip and
    the analytic floor B*(C-1)*1e-12 are applied host-side along with the
    final sum (host already owned the cross-core reduction).
  - sync rules kept from v1 (sim race detector + hardware):
      * every DMA whose completion matters gets its own semaphore;
      * SWDGE sems are never shared with HWDGE DMAs;
      * same-engine RAW on DVE gets an explicit sem edge (dve_sem chain).
"""

import hashlib
from contextlib import ExitStack

import ml_dtypes
import numpy as np

import concourse.bass as bass
from concourse import mybir
from concourse.bass_utils import run_bass_kernel_spmd

B = 4096
D = 512
C = 10000
NCORES = 8
BL = B // NCORES          # 512 rows per core
P = 128                   # partitions
NT = BL // P              # 4 chunks per core

CW = 520                  # bf16 cols per baked centers row (16B-aligned stride)
GW = D + 1                # gathered cols per row: center + (-|c|^2/2)
XW = D + 4                # f32 cols per x_ext chunk (col D = 1.0, 16B stride)

F32 = mybir.dt.float32
BF16 = mybir.dt.bfloat16
I32 = mybir.dt.int32

_CACHE = {}


def legalize_waits(nc, max_waits=1):
    """The walrus build in this container accepts at most one embedded
    sem-wait per TPB instruction ("Too many sync wait commands" otherwise).
    Split any excess into standalone single-wait InstEventSemaphore no-ops
    immediately before the instruction on the same engine — engine program
    order then enforces the identical synchronization."""
    n_split = 0
    for f in nc.m.functions:
        for b in f.blocks:
            insts = list(b.instructions)
            out = []
            for inst in insts:
                si = inst.sync_info
                waits = list(si.on_wait) if (si is not None and si.on_wait) else []
                if len(waits) > max_waits:
                    keep = waits[-max_waits:]
                    spill = waits[:-max_waits]
                    for k, w in enumerate(spill):
                        out.append(
                            mybir.InstEventSemaphore(
                                name=f"{inst.name}-lw{k}",
                                engine=inst.engine,
                                sync_info=mybir.SyncInfo(on_wait=[w], on_update=[]),
                            )
                        )
                        n_split += 1
                    inst.sync_info = mybir.SyncInfo(
                        on_wait=keep, on_update=list(si.on_update or [])
                    )
                out.append(inst)
            b.instructions = out
    return n_split


def make_caug(centers_np):
    """bf16 [C, CW]: cols 0..D-1 = centers, col D = -||c||^2/2, rest 0."""
    c64 = np.asarray(centers_np, dtype=np.float64)
    csq = (c64 * c64).sum(axis=1)
    caug = np.zeros((C, CW), dtype=np.float32)
    caug[:, :D] = centers_np
    caug[:, D] = (-0.5 * csq).astype(np.float32)
    return np.ascontiguousarray(caug.astype(ml_dtypes.bfloat16))


def build_nc(centers_np):
    nc = bass.Bass(num_swdge_queues=2, enable_partition_id=False)

    x = nc.dram_tensor("x", [BL, D], F32, kind="ExternalInput")
    # labels pre-arranged on host: [p, t] = original label[t*128 + p]
    labels = nc.dram_tensor("labels", [P, NT], I32, kind="ExternalInput")
    out = nc.dram_tensor("out", [P, NT], F32, kind="ExternalOutput")
    caug = nc.inline_tensor(make_caug(centers_np), name="caug")

    es = ExitStack()
    idx_sb = es.enter_context(nc.sbuf_tensor("idx_sb", [P, NT], I32))
    x_ext = es.enter_context(nc.sbuf_tensor("x_ext", [P, NT * XW], F32))
    c_sb = es.enter_context(nc.sbuf_tensor("c_sb", [P, NT * GW], BF16))
    sq_sb = es.enter_context(nc.sbuf_tensor("sq_sb", [P, NT * D], F32))
    prod_sb = es.enter_context(nc.sbuf_tensor("prod_sb", [P, NT * GW], F32))
    xsq_sb = es.enter_context(nc.sbuf_tensor("xsq_sb", [P, NT], F32))
    acc_sb = es.enter_context(nc.sbuf_tensor("acc_sb", [P, NT], F32))
    dist_sb = es.enter_context(nc.sbuf_tensor("dist_sb", [P, NT], F32))
    scr_sb = es.enter_context(nc.sbuf_tensor("scr_sb", [P, NT], I32))
    scr2_sb = es.enter_context(nc.sbuf_tensor("scr2_sb", [P, NT], I32))
    idx_sem = es.enter_context(nc.semaphore("idx_sem"))
    xc_sems = [es.enter_context(nc.semaphore(f"xc_sem{t}")) for t in range(NT)]
    c_sems = [es.enter_context(nc.semaphore(f"c_sem{t}")) for t in range(NT)]
    v_sem = es.enter_context(nc.semaphore("v_sem"))
    o_sem = es.enter_context(nc.semaphore("o_sem"))
    dve_sem = es.enter_context(nc.semaphore("dve_sem"))
    f_sem = es.enter_context(nc.semaphore("f_sem"))

    # ---- sync/HWDGE: labels first (they gate the gathers), then x chunks ----
    nc.sync.dma_start(out=idx_sb[:, :], in_=labels[:, :]).then_inc(idx_sem, 16)
    for t in range(NT):
        nc.sync.dma_start(
            out=x_ext[:, t * XW:t * XW + D], in_=x[t * P:(t + 1) * P, :]
        ).then_inc(xc_sems[t], 16)

    # ---- gpsimd: warm-up dummies on both SWDGE queues while the label DMA
    # is in flight (the first SWDGE DMA after engine idle pays ~1 us of
    # warm-up; absorb it in the dead time), then the four gathers ----
    nc.gpsimd.dma_start(out=scr_sb[:, :], in_=labels[:, :]).then_inc(f_sem, 16)
    w1 = nc.gpsimd.dma_start(out=scr2_sb[:, :], in_=labels[:, :]).then_inc(f_sem, 16)
    w1.ins.queue = "qPoolDynamic1"
    # ---- four 128-row gathers alternating across SWDGE queues ----
    nc.gpsimd.wait_ge(idx_sem, 16)
    gather_insts = []
    for t in range(NT):
        gi = nc.gpsimd.indirect_dma_start(
            out=c_sb[:, t * GW:(t + 1) * GW],
            out_offset=None,
            in_=caug[:],
            in_offset=bass.IndirectOffsetOnAxis(ap=idx_sb[:, t:t + 1], axis=0),
        ).then_inc(c_sems[t], 16)
        if t % 2 == 1:
            gi.ins.queue = "qPoolDynamic1"
        gather_insts.append(gi)
    # trailing dummy SWDGE DMAs: flush each queue's gather completion receipt
    nc.gpsimd.dma_start(out=scr_sb[:, :], in_=labels[:, :]).then_inc(f_sem, 16)
    d1 = nc.gpsimd.dma_start(out=scr2_sb[:, :], in_=labels[:, :]).then_inc(f_sem, 16)
    d1.ins.queue = "qPoolDynamic1"

    # ---- vector (DVE) ----
    n_dve = 0
    # x_ext's 1.0 column (disjoint from the DMA'd cols, no ordering needed)
    for t in range(NT):
        nc.vector.memset(x_ext[:, t * XW + D:t * XW + D + 1], 1.0).then_inc(dve_sem, 1)
        n_dve += 1
    # ||x||^2 per chunk while the gathers are still in flight
    for t in range(NT):
        xc = x_ext[:, t * XW:t * XW + D]
        nc.vector.wait_ge(xc_sems[t], 16)
        nc.vector.scalar_tensor_tensor(
            out=sq_sb[:, t * D:(t + 1) * D],
            in0=xc,
            scalar=1.0,
            in1=xc,
            op0=mybir.AluOpType.mult,
            op1=mybir.AluOpType.mult,
            accum_out=xsq_sb[:, t:t + 1],
        ).then_inc(dve_sem, 1)
        n_dve += 1
    # fused per-chunk partial distance: acc = sum(-2 * c_aug * x_ext)
    #                                       = ||c||^2 - 2<x, c>
    n_pre = n_dve  # memsets + squares done (RAW edges for x_ext col D)
    for t in range(NT):
        nc.vector.wait_ge(c_sems[t], 16)
        nc.vector.wait_ge(dve_sem, n_pre)
        nc.vector.scalar_tensor_tensor(
            out=prod_sb[:, t * GW:(t + 1) * GW],
            in0=c_sb[:, t * GW:(t + 1) * GW],
            scalar=-2.0,
            in1=x_ext[:, t * XW:t * XW + GW],
            op0=mybir.AluOpType.mult,
            op1=mybir.AluOpType.mult,
            accum_out=acc_sb[:, t:t + 1],
        ).then_inc(dve_sem, 1)
        n_dve += 1
    # dist = acc + ||x||^2
    nc.vector.wait_ge(dve_sem, n_dve)
    nc.vector.tensor_tensor(
        out=dist_sb[:, :],
        in0=acc_sb[:, :],
        in1=xsq_sb[:, :],
        op=mybir.AluOpType.add,
    ).then_inc(v_sem, 1)

    # ---- result out via the idle sync/HWDGE queue ----
    nc.sync.wait_ge(v_sem, 1)
    nc.sync.dma_start(out=out[:, :], in_=dist_sb[:, :]).then_inc(o_sem, 16)

    # NOTE: the ExitStack is intentionally NOT closed — closing would free
    # the semaphores and emit an expensive end-of-program drain + barrier;
    # the NEFF-level postamble already clears the kernel sem range, so
    # repeated executions stay safe without it.
    legalize_waits(nc)
    return nc


def _get_nc(centers_np):
    arr = np.ascontiguousarray(centers_np, np.float32)
    key = hashlib.md5(arr.tobytes()).hexdigest()
    if _CACHE.get("key") != key:
        _CACHE["nc"] = build_nc(arr)
        _CACHE["key"] = key
    return _CACHE["nc"]


def make_in_maps(x, labels, centers=None):
    x = np.ascontiguousarray(np.asarray(x, dtype=np.float32))
    # [p, t] = label[t*128 + p] within each core's 512-row shard
    labels_i32 = np.ascontiguousarray(
        np.asarray(labels).astype(np.int32).reshape(NCORES, NT, P).transpose(0, 2, 1)
    )
    xs = x.reshape(NCORES, BL, D)
    return [{"x": xs[i], "labels": labels_i32[i]} for i in range(NCORES)]


def finalize(results):
    total = 0.0
    for r in results:
        d = np.asarray(r["out"], dtype=np.float64)
        total += float(np.clip(d, 1e-12, 1e12).sum())
    loss = (total + B * (C - 1) * 1e-12) / B
    return np.array(loss, dtype=np.float32)


def kernel(x, labels, centers):
    nc = _get_nc(centers)
    in_maps = make_in_maps(x, labels)
    res = run_bass_kernel_spmd(nc, in_maps, core_ids=list(range(NCORES)))
    return finalize(res.results)
